# revision 32
# baseline (speedup 1.0000x reference)
"""Multi-head attention (B=2, S=2048, D=1024, H=16, Dh=64) on 8 Trainium2
NeuronCores via Bass/Tile.

Sharding: core c = 4*b + g handles batch b and head group g (4 heads =
2 "pairs" of 64-dim heads stacked on the 128-partition dim), with the
matching column/row slices of Wq/Wk/Wv/Wo. Each core returns its partial
output projection; the host sums the 4 partials per batch and adds bo.

Design notes:
  * Host pre-transposes + pre-casts x and the weight slices to bf16 in the
    exact SBUF layouts (no on-device PE transposes, no casting DMAs).
  * The key axis is compacted on host to the unmasked keys (padded to a
    whole number of 128-key chunks): scores/ctx/K-proj/V-proj matmuls and
    the exp() stream all shrink by the masked fraction. Pad keys get V=0
    and a 0 in the denominator column, so no mask arithmetic on device.
  * The attention kc loop is software-pipelined (scores for kc+1 issued
    before ctx for kc) and all remaining projection / output work is
    diced into 2-matmul "filler chunks" popped one (or two) per kc so the
    PE stays busy at the exp-paced pipeline rate without starving the ACT
    engine.
  * Normalization is deferred: ctx PSUM (with the denominator row) is
    evacuated to SBUF with plain copies to free the PSUM bank quickly;
    reciprocal/broadcast/multiply then run off the critical PE path.
  * Weights load on the scalar engine's DMA queue in parallel with x
    tiles on the sync queue; small tensors ride the gpsimd queue.

Per-core math: QT/KT = W^T x^T + b in [dh, s] layout, V_ext =
[(x_kept Wv + bv) * keepmask | keepmask] per head; per q-tile & key chunk:
scT = KT_chunk^T QT_tile (2 heads row-packed in the PE), eT =
exp(SCALE * scT) (one ACT op per pair), ctx_h[65, q] += V_ext^T eT_h
(row 64 = softmax denominator); normalize via reciprocal+broadcast; then
out_partial = ctxT^T Wo_g accumulated over the 2 pairs.
"""

import itertools
import math
from collections import deque

import ml_dtypes
import numpy as np

import concourse.bacc as bacc
import concourse.mybir as mybir
import concourse.tile as tile
from concourse.bass_utils import run_bass_kernel_spmd

F32 = mybir.dt.float32
BF16 = mybir.dt.bfloat16
AF = mybir.ActivationFunctionType
NPBF16 = ml_dtypes.bfloat16

S = 2048
D = 1024
HPC = 4                  # heads per core
DH = 64
PAIRS = 2                # head pairs per core
P = 128
QW = 512                 # q tile width
QT_TILES = S // QW       # 4
DCH = D // P             # 8
SCALE = 1.0 / math.sqrt(DH)

N_CORES = 8


def build(nkc):
    """Build the per-core kernel for `nkc` 128-key chunks of kept keys."""
    _uid = itertools.count()
    NK = nkc * P
    nc = bacc.Bacc(None, target_bir_lowering=False, num_swdge_queues=4)

    xt = nc.dram_tensor("xt", [QT_TILES, P, DCH, QW], BF16, kind="ExternalInput")
    KBS = 3 if nkc % 3 == 0 else 1      # key-block size in 128-key chunks
    NKB = nkc // KBS
    KW = KBS * P
    xtk = nc.dram_tensor("xtk", [NKB, P, DCH, KW], BF16, kind="ExternalInput")
    wq = nc.dram_tensor("wq", [P, DCH, 2 * P], BF16, kind="ExternalInput")
    wk = nc.dram_tensor("wk", [P, DCH, 2 * P], BF16, kind="ExternalInput")
    wv = nc.dram_tensor("wv", [P, DCH, 2 * P], BF16, kind="ExternalInput")
    wo = nc.dram_tensor("wo", [P, PAIRS, D], BF16, kind="ExternalInput")
    bq = nc.dram_tensor("bq", [P, PAIRS], F32, kind="ExternalInput")
    bk = nc.dram_tensor("bk", [P, PAIRS], F32, kind="ExternalInput")
    bvB = nc.dram_tensor("bvB", [P, 2 * P], F32, kind="ExternalInput")
    mcol = nc.dram_tensor("mcol", [P, nkc], F32, kind="ExternalInput")
    out = nc.dram_tensor("out", [S, D], BF16, kind="ExternalOutput")

    with tile.TileContext(nc) as tc:
        with (
            tc.tile_pool(name="persist", bufs=1) as pp,
            tc.tile_pool(name="vstage", bufs=3) as xs,
            tc.tile_pool(name="expp", bufs=3) as ep,
            tc.tile_pool(name="smalls", bufs=4) as sp,
            tc.tile_pool(name="craws", bufs=2) as cw,
            tc.tile_pool(name="ps_sc", bufs=2, space="PSUM") as ps_sc,
            tc.tile_pool(name="ps_ctx", bufs=2, space="PSUM") as ps_ctx,
            tc.tile_pool(name="ps_w", bufs=2, space="PSUM") as ps_w,
        ):
            # ---- persistent SBUF tensors ----
            wq_sb = pp.tile([P, DCH, 2 * P], BF16)
            wk_sb = pp.tile([P, DCH, 2 * P], BF16)
            wv_sb = pp.tile([P, DCH, 2 * P], BF16)
            wo_sb = pp.tile([P, PAIRS, D], BF16)
            xt_sb = pp.tile([P, QT_TILES, DCH, QW], BF16)
            xtk_sb = pp.tile([P, NKB, DCH, KW], BF16)
            bq_sb = pp.tile([P, PAIRS], F32)
            bk_sb = pp.tile([P, PAIRS], F32)
            bvB_sb = pp.tile([P, 2 * P], F32)
            mcol_sb = pp.tile([P, nkc], F32)

            QT = pp.tile([P, PAIRS, S], BF16)
            KT = pp.tile([P, PAIRS, NK], BF16)
            VE = pp.tile([P, nkc, HPC * (DH + 1)], BF16)
            ctxq = [pp.tile([P, PAIRS, QW], BF16, name=f"ctxq{i}")
                    for i in range(QT_TILES)]

            # ---- input DMAs: smalls on gpsimd, weights on the scalar
            # engine's queue, x tiles on sync (FIFO = priority) ----
            nc.gpsimd.dma_start(mcol_sb[:], mcol[:])
            nc.gpsimd.dma_start(bvB_sb[:], bvB[:])
            nc.gpsimd.dma_start(bq_sb[:], bq[:])
            nc.gpsimd.dma_start(bk_sb[:], bk[:])
            nc.scalar.dma_start(wv_sb[:], wv[:])
            nc.scalar.dma_start(wk_sb[:], wk[:])
            nc.scalar.dma_start(wq_sb[:], wq[:])
            nc.scalar.dma_start(wo_sb[:], wo[:])
            for wi in range(NKB):
                nc.sync.dma_start(xtk_sb[:, wi, :, :], xtk[wi])
            for qt in range(QT_TILES):
                nc.sync.dma_start(xt_sb[:, qt, :, :], xt[qt])

            # keep-mask (1=kept, 0=pad) into the denominator columns of V_ext
            ve4 = VE[:].rearrange("p k (h c) -> p k h c", h=HPC)
            nc.vector.tensor_copy(
                ve4[:, :, :, DH : DH + 1],
                mcol_sb[:, :, None, None].to_broadcast([P, nkc, HPC, 1]),
            )

            # ---- work units, diced into 2-matmul chunks ----
            def v_chunks(st):
                box = []

                def mk(dcs, final):
                    def emit():
                        if not box:
                            box.append(ps_w.tile([P, QW], F32, tag="w", name=f"w{next(_uid)}"))
                        pv = box[0]
                        blk, off = divmod(st, KBS)
                        for dc in dcs:
                            nc.tensor.matmul(
                                pv[:, : 2 * P],
                                xtk_sb[:, blk, dc, off * P : (off + 1) * P],
                                wv_sb[:, dc, :],
                                start=(dc == 0),
                                stop=(dc == DCH - 1),
                            )
                        if final:
                            vtmp = xs.tile([P, 2 * P], F32, tag="vtmp", name=f"vt{next(_uid)}")
                            nc.vector.tensor_add(
                                vtmp[:], pv[:, : 2 * P], bvB_sb[:]
                            )
                            nc.vector.tensor_scalar_mul(
                                ve4[:, st, :, 0:DH],
                                vtmp[:].rearrange("p (h c) -> p h c", h=HPC),
                                mcol_sb[:, st : st + 1],
                            )

                    return emit

                return [mk([0, 1], False), mk([2, 3], False),
                        mk([4, 5], False), mk([6, 7], True)]

            def kq_chunks(dst, dsl, src, w_sb, b_sb, pr, width):
                # dst[:, pr, dsl] (width wide) = W_pr^T @ src(dc) + b
                box = []

                def mk(dcs, final):
                    def emit():
                        if not box:
                            box.append(ps_w.tile([P, QW], F32, tag="w", name=f"w{next(_uid)}"))
                        pq = box[0]
                        for dc in dcs:
                            nc.tensor.matmul(
                                pq[:, :width],
                                w_sb[:, dc, pr * P : (pr + 1) * P],
                                src(dc),
                                start=(dc == 0),
                                stop=(dc == DCH - 1),
                            )
                        if final:
                            nc.vector.tensor_scalar_add(
                                dst[:, pr, dsl],
                                pq[:, :width],
                                b_sb[:, pr : pr + 1],
                            )

                    return emit

                return [mk([0, 1], False), mk([2, 3], False),
                        mk([4, 5], False), mk([6, 7], True)]

            def q_chunks(pr, qt):
                return kq_chunks(
                    QT, slice(qt * QW, (qt + 1) * QW),
                    lambda dc: xt_sb[:, qt, dc, :],
                    wq_sb, bq_sb, pr, QW,
                )

            def k_chunks(pr, blk):
                return kq_chunks(
                    KT, slice(blk * KW, (blk + 1) * KW),
                    lambda dc: xtk_sb[:, blk, dc, :],
                    wk_sb, bk_sb, pr, KW,
                )

            def out_chunks(st, tail=False):
                box = []

                def mk(nt):
                    def emit():
                        if not box:
                            box.append(xs.tile([P, D], BF16, tag="ob", name=f"ob{next(_uid)}"))
                        ob = box[0]
                        po = ps_w.tile([P, QW], F32, tag="w", name=f"w{next(_uid)}")
                        oqt, ooff = divmod(st, 4)
                        for pr in range(PAIRS):
                            nc.tensor.matmul(
                                po[:],
                                ctxq[oqt][:, pr, ooff * P : (ooff + 1) * P],
                                wo_sb[:, pr, nt * QW : (nt + 1) * QW],
                                start=(pr == 0),
                                stop=(pr == PAIRS - 1),
                            )
                        osl = slice(nt * QW, (nt + 1) * QW)
                        if tail and (st + nt) % 2 == 1:
                            nc.scalar.copy(ob[:, osl], po[:])
                        else:
                            nc.vector.tensor_copy(ob[:, osl], po[:])
                        if tail:
                            nc.sync.dma_start(
                                out[st * P : (st + 1) * P, osl], ob[:, osl]
                            )
                        elif nt == 1:
                            nc.sync.dma_start(
                                out[st * P : (st + 1) * P, :], ob[:]
                            )

                    return emit

                return [mk(0), mk(1)]

            sched = [(pr, qt, 0, QW)
                     for qt in range(QT_TILES) for pr in (0, 1)]
            sc_stream = [(pr, qt, kc, q0, qw)
                         for (pr, qt, q0, qw) in sched for kc in range(nkc)]
            sc_pos = [0]
            et_map = {}

            def emit_next_sc():
                if sc_pos[0] >= len(sc_stream):
                    return
                pr, qt, kc, q0, qw = sc_stream[sc_pos[0]]
                sc_pos[0] += 1
                qsl = slice(qt * QW + q0, qt * QW + q0 + qw)
                sc = ps_sc.tile([P, 2, QW], F32, tag="sc", name=f"sc{next(_uid)}")
                for hh in range(2):
                    nc.tensor.matmul(
                        sc[:, hh, :qw],
                        KT[hh * DH : (hh + 1) * DH, pr, kc * P : (kc + 1) * P],
                        QT[hh * DH : (hh + 1) * DH, pr, qsl],
                        start=True,
                        stop=True,
                        tile_position=(hh * DH, 0),
                    )
                et = ep.tile([P, 2, QW], BF16, tag="et", name=f"et{next(_uid)}")
                nc.scalar.activation(et[:, :, :qw], sc[:, :, :qw],
                                     AF.Exp, scale=float(SCALE))
                et_map[(pr, qt, kc, q0)] = et

            def attention(pr, qt, fillers, max_pops_per_kc, q0=0, qw=QW,
                          last=False):
                cps = [
                    ps_ctx.tile([DH + 1, QW], F32, tag="ctx", name=f"ctx{hh}")
                    for hh in range(2)
                ]
                budget = min(len(fillers), max_pops_per_kc * nkc)
                if max_pops_per_kc == 1:
                    # keep spare (non-critical) fillers for later attentions,
                    # but always drain through the last correctness-critical
                    # chunk (its write must be emitted before its reader)
                    last_crit = -1
                    for i, (_, crit) in enumerate(fillers):
                        if crit:
                            last_crit = i
                    budget = min(budget, max(6, last_crit + 1))
                popped = 0
                for kc in range(nkc):
                    target = ((kc + 1) * budget + nkc - 1) // nkc
                    while popped < target and fillers:
                        fillers.popleft()[0]()
                        popped += 1
                    emit_next_sc()
                    et = et_map.pop((pr, qt, kc, q0))
                    for hh in range(2):
                        h = 2 * pr + hh
                        nc.tensor.matmul(
                            cps[hh][: DH + 1, :qw],
                            VE[:, kc, h * (DH + 1) : (h + 1) * (DH + 1)],
                            et[:, hh, :qw],
                            start=(kc == 0),
                            stop=(kc == nkc - 1),
                        )
                # fast PSUM evacuation (frees ctx banks), then deferred
                # normalize off the PE critical path. The final attention has
                # no successor waiting on the banks: normalize straight from
                # PSUM and skip the evacuation hop.
                if last:
                    recBs = []
                    for hh in range(2):
                        den = sp.tile([1, QW], F32, tag="den", name=f"den{hh}")
                        nc.vector.tensor_copy(den[:, :qw],
                                              cps[hh][DH : DH + 1, :qw])
                        rec = sp.tile([1, QW], F32, tag="rec", name=f"rec{hh}")
                        nc.vector.reciprocal_approx_fast(rec[:, :qw], den[:, :qw])
                        recB = sp.tile([DH, QW], F32, tag="recB",
                                       name=f"recB{hh}")
                        nc.gpsimd.partition_broadcast(recB[:, :qw], rec[:, :qw])
                        recBs.append(recB)
                    for hh in range(2):
                        nc.vector.tensor_mul(
                            ctxq[qt][hh * DH : (hh + 1) * DH, pr, q0 : q0 + qw],
                            cps[hh][:DH, :qw],
                            recBs[hh][:, :qw],
                        )
                    return
                craws = []
                for hh in range(2):
                    craw = cw.tile([DH + 1, QW], F32, tag="craw",
                                   name=f"craw{hh}")
                    nc.vector.tensor_copy(craw[:, :qw], cps[hh][:, :qw])
                    craws.append(craw)
                for hh in range(2):
                    craw = craws[hh]
                    den = sp.tile([1, QW], F32, tag="den", name=f"den{hh}")
                    nc.vector.tensor_copy(den[:, :qw], craw[DH : DH + 1, :qw])
                    rec = sp.tile([1, QW], F32, tag="rec", name=f"rec{hh}")
                    nc.vector.reciprocal_approx_fast(rec[:, :qw], den[:, :qw])
                    recB = sp.tile([DH, QW], F32, tag="recB", name=f"recB{hh}")
                    nc.gpsimd.partition_broadcast(recB[:, :qw], rec[:, :qw])
                    nc.vector.tensor_mul(
                        ctxq[qt][hh * DH : (hh + 1) * DH, pr, q0 : q0 + qw],
                        craw[:DH, :qw],
                        recB[:, :qw],
                    )

            # ---- emission (scheduling priority) ----
            import os
            _NOFILL = os.environ.get("KMOD_NOFILL") == "1"
            N_PRE_V = nkc if _NOFILL else min(12, nkc)
            for st in range(N_PRE_V):
                for ch in v_chunks(st):
                    ch()
            k_blks = list(range(NKB))
            n_pre_k = NKB if _NOFILL else (2 if KBS == 3 else 4)
            for blk in k_blks[:n_pre_k]:
                for ch in k_chunks(0, blk):
                    ch()
            for ch in q_chunks(0, 0):
                ch()
            for ch in q_chunks(1, 0):
                ch()

            # K(pair0) blocks first (sc(0,0,kc) consumes block kc//KBS with a
            # one-iteration emission lookahead), then trailing V (consumed at
            # ctx(st)), then K(pair1) for the second attention
            def crit(chs):
                return [(f, True) for f in chs]

            def noncrit(chs):
                return [(f, False) for f in chs]

            fillers = deque()
            for blk in k_blks[n_pre_k:]:
                fillers.extend(crit(k_chunks(0, blk)))
            for st in range(N_PRE_V, nkc):
                fillers.extend(crit(v_chunks(st)))
            for blk in k_blks:
                fillers.extend(crit(k_chunks(1, blk)))

            def drain():
                while fillers:
                    fillers.popleft()[0]()

            emit_next_sc()
            for qt in range(QT_TILES):
                if _NOFILL:
                    drain()
                mp0 = -(-len(fillers) // nkc) if qt == 0 else 1
                attention(0, qt, fillers, 0 if _NOFILL else mp0)
                if qt + 1 < QT_TILES:
                    fillers.extend(crit(q_chunks(0, qt + 1)))
                if qt >= 1:
                    for st in ((4 * qt - 2, 4 * qt - 1) if qt < 3 else (10, 11)):
                        fillers.extend(noncrit(out_chunks(st)))
                if _NOFILL:
                    drain()
                attention(1, qt, fillers, 0 if _NOFILL else 1,
                          last=(qt == QT_TILES - 1))
                if qt + 1 < QT_TILES:
                    fillers.extend(crit(q_chunks(1, qt + 1)))
                    for st in (4 * qt, 4 * qt + 1):
                        fillers.extend(noncrit(out_chunks(st)))
            for st in (12, 13, 14, 15):
                fillers.extend(noncrit(out_chunks(st, tail=True)))
            drain()

    nc.finalize()
    return nc


def shard_inputs(x, Wq, bq, Wk, bk, Wv, bv, Wo, bo, mask):
    """Full inputs -> (nkc, list of 8 per-core input maps)."""
    x = np.asarray(x, dtype=np.float32)
    mask = np.asarray(mask)
    kept = [np.flatnonzero(~mask[b]) for b in range(2)]
    nkc = max(1, max((len(k) + P - 1) // P for k in kept))
    NK = nkc * P
    KBS = 3 if nkc % 3 == 0 else 1
    NKB = nkc // KBS
    KW = KBS * P

    def to_T_blocked(a):
        # [rows, cols(=n*128)] fp32 -> [128, n, rows] bf16 with
        # out[p, c, r] = a[r, c*128+p]
        rows, cols = a.shape
        n = cols // P
        return np.ascontiguousarray(
            a.T.astype(NPBF16).reshape(n, P, rows).transpose(1, 0, 2)
        )

    per_batch = {}
    for b in range(2):
        idx = kept[b]
        xk = np.zeros((NK, D), dtype=np.float32)
        xk[: len(idx)] = x[b][idx]
        mc = np.zeros((NK,), dtype=np.float32)
        mc[: len(idx)] = 1.0
        xt_t = to_T_blocked(x[b])        # [P, DCH, S]
        xtk_t = to_T_blocked(xk)         # [P, DCH, NK]
        per_batch[b] = {
            "xt": np.ascontiguousarray(
                xt_t.reshape(P, DCH, QT_TILES, QW).transpose(2, 0, 1, 3)
            ),
            "xtk": np.ascontiguousarray(
                xtk_t.reshape(P, DCH, NKB, KW).transpose(2, 0, 1, 3)
            ),
            "mcol": np.ascontiguousarray(mc.reshape(nkc, P).T),
        }

    ins = []
    for c in range(N_CORES):
        b, g = divmod(c, 4)
        cs = slice(g * 256, (g + 1) * 256)
        wq_h = np.ascontiguousarray(
            Wq[:, cs].astype(NPBF16).reshape(DCH, P, 2 * P).transpose(1, 0, 2)
        )
        wk_h = np.ascontiguousarray(
            Wk[:, cs].astype(NPBF16).reshape(DCH, P, 2 * P).transpose(1, 0, 2)
        )
        wv_h = np.ascontiguousarray(
            Wv[:, cs].astype(NPBF16).reshape(DCH, P, 2 * P).transpose(1, 0, 2)
        )
        wo_h = np.ascontiguousarray(
            Wo[cs, :].astype(NPBF16).reshape(PAIRS, P, D).transpose(1, 0, 2)
        )
        ins.append(
            {
                **per_batch[b],
                "wq": wq_h,
                "wk": wk_h,
                "wv": wv_h,
                "wo": wo_h,
                "bq": np.ascontiguousarray(
                    np.asarray(bq[cs], dtype=np.float32).reshape(PAIRS, P).T
                ),
                "bk": np.ascontiguousarray(
                    np.asarray(bk[cs], dtype=np.float32).reshape(PAIRS, P).T
                ),
                "bvB": np.ascontiguousarray(
                    np.tile(np.asarray(bv[cs], dtype=np.float32)[None, :], (P, 1))
                ),
            }
        )
    return nkc, ins


def gather_outputs(results, bo):
    """8 per-core partial outputs -> full (2, S, D) fp32 output."""
    outs = []
    for b in range(2):
        acc = results[4 * b]["out"].astype(np.float32).copy()
        for g in range(1, 4):
            acc += results[4 * b + g]["out"]
        outs.append(acc + np.asarray(bo, dtype=np.float32))
    return np.stack(outs, axis=0)


_NC_CACHE = {}


def _get_nc(nkc):
    if nkc not in _NC_CACHE:
        _NC_CACHE[nkc] = build(nkc)
    return _NC_CACHE[nkc]


def run_sharded(inputs, trace=False, tmpdir=None):
    """Shard, run on cores 0-7, gather. Returns (output, BassKernelResults)."""
    nkc, ins = shard_inputs(**inputs)
    nc = _get_nc(nkc)
    res = run_bass_kernel_spmd(
        nc, ins, core_ids=list(range(N_CORES)), trace=trace, tmpdir=tmpdir
    )
    full = gather_outputs(res.results, inputs["bo"])
    return full, res


def kernel(**inputs) -> np.ndarray:
    full, _ = run_sharded(inputs, trace=False)
    return full


# revision 33
# speedup vs baseline: 1.0157x; 1.0157x over previous
"""Multi-head attention (B=2, S=2048, D=1024, H=16, Dh=64) on 8 Trainium2
NeuronCores via Bass/Tile.

Sharding: core c = 4*b + g handles batch b and head group g (4 heads =
2 "pairs" of 64-dim heads stacked on the 128-partition dim), with the
matching column/row slices of Wq/Wk/Wv/Wo. Each core returns its partial
output projection; the host sums the 4 partials per batch and adds bo.

Design notes:
  * Host pre-transposes + pre-casts x and the weight slices to bf16 in the
    exact SBUF layouts (no on-device PE transposes, no casting DMAs).
  * The key axis is compacted on host to the unmasked keys (padded to a
    whole number of 128-key chunks): scores/ctx/K-proj/V-proj matmuls and
    the exp() stream all shrink by the masked fraction. Pad keys get V=0
    and a 0 in the denominator column, so no mask arithmetic on device.
  * The attention kc loop is software-pipelined (scores for kc+1 issued
    before ctx for kc) and all remaining projection / output work is
    diced into 2-matmul "filler chunks" popped one (or two) per kc so the
    PE stays busy at the exp-paced pipeline rate without starving the ACT
    engine.
  * Normalization is deferred: ctx PSUM (with the denominator row) is
    evacuated to SBUF with plain copies to free the PSUM bank quickly;
    reciprocal/broadcast/multiply then run off the critical PE path.
  * Weights load on the scalar engine's DMA queue in parallel with x
    tiles on the sync queue; small tensors ride the gpsimd queue.

Per-core math: QT/KT = W^T x^T + b in [dh, s] layout, V_ext =
[(x_kept Wv + bv) * keepmask | keepmask] per head; per q-tile & key chunk:
scT = KT_chunk^T QT_tile (2 heads row-packed in the PE), eT =
exp(SCALE * scT) (one ACT op per pair), ctx_h[65, q] += V_ext^T eT_h
(row 64 = softmax denominator); normalize via reciprocal+broadcast; then
out_partial = ctxT^T Wo_g accumulated over the 2 pairs.
"""

import itertools
import math
from collections import deque

import ml_dtypes
import numpy as np

import concourse.bacc as bacc
import concourse.mybir as mybir
import concourse.tile as tile
from concourse.bass_utils import run_bass_kernel_spmd

F32 = mybir.dt.float32
BF16 = mybir.dt.bfloat16
AF = mybir.ActivationFunctionType
NPBF16 = ml_dtypes.bfloat16

S = 2048
D = 1024
HPC = 4                  # heads per core
DH = 64
PAIRS = 2                # head pairs per core
P = 128
QW = 512                 # q tile width
QT_TILES = S // QW       # 4
DCH = D // P             # 8
SCALE = 1.0 / math.sqrt(DH)

N_CORES = 8


def build(nkc):
    """Build the per-core kernel for `nkc` 128-key chunks of kept keys."""
    _uid = itertools.count()
    NK = nkc * P
    nc = bacc.Bacc(None, target_bir_lowering=False, num_swdge_queues=4)

    xt = nc.dram_tensor("xt", [QT_TILES, P, DCH, QW], BF16, kind="ExternalInput")
    KBS = 3 if nkc % 3 == 0 else 1      # key-block size in 128-key chunks
    NKB = nkc // KBS
    KW = KBS * P
    xtk = nc.dram_tensor("xtk", [NKB, P, DCH, KW], BF16, kind="ExternalInput")
    wq = nc.dram_tensor("wq", [P, DCH, 2 * P], BF16, kind="ExternalInput")
    wk = nc.dram_tensor("wk", [P, DCH, 2 * P], BF16, kind="ExternalInput")
    wv = nc.dram_tensor("wv", [P, DCH, 2 * P], BF16, kind="ExternalInput")
    wo = nc.dram_tensor("wo", [P, PAIRS, D], BF16, kind="ExternalInput")
    bq = nc.dram_tensor("bq", [P, PAIRS], F32, kind="ExternalInput")
    bk = nc.dram_tensor("bk", [P, PAIRS], F32, kind="ExternalInput")
    bvB = nc.dram_tensor("bvB", [P, 2 * P], F32, kind="ExternalInput")
    mcol = nc.dram_tensor("mcol", [P, nkc], F32, kind="ExternalInput")
    out = nc.dram_tensor("out", [S, D], BF16, kind="ExternalOutput")

    with tile.TileContext(nc) as tc:
        with (
            tc.tile_pool(name="persist", bufs=1) as pp,
            tc.tile_pool(name="vstage", bufs=4) as xs,
            tc.tile_pool(name="expp", bufs=4) as ep,
            tc.tile_pool(name="smalls", bufs=6) as sp,
            tc.tile_pool(name="craws", bufs=2) as cw,
            tc.tile_pool(name="ps_sc", bufs=2, space="PSUM") as ps_sc,
            tc.tile_pool(name="ps_ctx", bufs=2, space="PSUM") as ps_ctx,
            tc.tile_pool(name="ps_w", bufs=2, space="PSUM") as ps_w,
        ):
            # ---- persistent SBUF tensors ----
            wq_sb = pp.tile([P, DCH, 2 * P], BF16)
            wk_sb = pp.tile([P, DCH, 2 * P], BF16)
            wv_sb = pp.tile([P, DCH, 2 * P], BF16)
            wo_sb = pp.tile([P, PAIRS, D], BF16)
            xt_sb = pp.tile([P, QT_TILES, DCH, QW], BF16)
            xtk_sb = pp.tile([P, NKB, DCH, KW], BF16)
            bq_sb = pp.tile([P, PAIRS], F32)
            bk_sb = pp.tile([P, PAIRS], F32)
            bvB_sb = pp.tile([P, 2 * P], F32)
            mcol_sb = pp.tile([P, nkc], F32)

            QT = pp.tile([P, PAIRS, S], BF16)
            KT = pp.tile([P, PAIRS, NK], BF16)
            VE = pp.tile([P, nkc, HPC * (DH + 1)], BF16)
            ctxq = [pp.tile([P, PAIRS, QW], BF16, name=f"ctxq{i}")
                    for i in range(QT_TILES)]

            # ---- input DMAs: smalls on gpsimd, weights on the scalar
            # engine's queue, x tiles on sync (FIFO = priority) ----
            nc.gpsimd.dma_start(mcol_sb[:], mcol[:])
            nc.gpsimd.dma_start(bvB_sb[:], bvB[:])
            nc.gpsimd.dma_start(bq_sb[:], bq[:])
            nc.gpsimd.dma_start(bk_sb[:], bk[:])
            nc.scalar.dma_start(wv_sb[:], wv[:])
            nc.scalar.dma_start(wk_sb[:], wk[:])
            nc.scalar.dma_start(wq_sb[:], wq[:])
            nc.scalar.dma_start(wo_sb[:], wo[:])
            for wi in range(NKB):
                nc.sync.dma_start(xtk_sb[:, wi, :, :], xtk[wi])
            for qt in range(QT_TILES):
                nc.sync.dma_start(xt_sb[:, qt, :, :], xt[qt])

            # keep-mask (1=kept, 0=pad) into the denominator columns of V_ext
            ve4 = VE[:].rearrange("p k (h c) -> p k h c", h=HPC)
            nc.vector.tensor_copy(
                ve4[:, :, :, DH : DH + 1],
                mcol_sb[:, :, None, None].to_broadcast([P, nkc, HPC, 1]),
            )

            # ---- work units, diced into 2-matmul chunks ----
            def v_chunks(st):
                box = []

                def mk(dcs, final):
                    def emit():
                        if not box:
                            box.append(ps_w.tile([P, QW], F32, tag="w", name=f"w{next(_uid)}"))
                        pv = box[0]
                        blk, off = divmod(st, KBS)
                        for dc in dcs:
                            nc.tensor.matmul(
                                pv[:, : 2 * P],
                                xtk_sb[:, blk, dc, off * P : (off + 1) * P],
                                wv_sb[:, dc, :],
                                start=(dc == 0),
                                stop=(dc == DCH - 1),
                            )
                        if final:
                            vtmp = xs.tile([P, 2 * P], F32, tag="vtmp", name=f"vt{next(_uid)}")
                            nc.vector.tensor_add(
                                vtmp[:], pv[:, : 2 * P], bvB_sb[:]
                            )
                            nc.vector.tensor_scalar_mul(
                                ve4[:, st, :, 0:DH],
                                vtmp[:].rearrange("p (h c) -> p h c", h=HPC),
                                mcol_sb[:, st : st + 1],
                            )

                    return emit

                return [mk([0, 1], False), mk([2, 3], False),
                        mk([4, 5], False), mk([6, 7], True)]

            def kq_chunks(dst, dsl, src, w_sb, b_sb, pr, width):
                # dst[:, pr, dsl] (width wide) = W_pr^T @ src(dc) + b
                box = []

                def mk(dcs, final):
                    def emit():
                        if not box:
                            box.append(ps_w.tile([P, QW], F32, tag="w", name=f"w{next(_uid)}"))
                        pq = box[0]
                        for dc in dcs:
                            nc.tensor.matmul(
                                pq[:, :width],
                                w_sb[:, dc, pr * P : (pr + 1) * P],
                                src(dc),
                                start=(dc == 0),
                                stop=(dc == DCH - 1),
                            )
                        if final:
                            nc.vector.tensor_scalar_add(
                                dst[:, pr, dsl],
                                pq[:, :width],
                                b_sb[:, pr : pr + 1],
                            )

                    return emit

                return [mk([0, 1], False), mk([2, 3], False),
                        mk([4, 5], False), mk([6, 7], True)]

            def q_chunks(pr, qt):
                return kq_chunks(
                    QT, slice(qt * QW, (qt + 1) * QW),
                    lambda dc: xt_sb[:, qt, dc, :],
                    wq_sb, bq_sb, pr, QW,
                )

            def k_chunks(pr, blk):
                return kq_chunks(
                    KT, slice(blk * KW, (blk + 1) * KW),
                    lambda dc: xtk_sb[:, blk, dc, :],
                    wk_sb, bk_sb, pr, KW,
                )

            def out_chunks(st, tail=False):
                box = []

                def mk(nt):
                    def emit():
                        if not box:
                            box.append(xs.tile([P, D], BF16, tag="ob", name=f"ob{next(_uid)}"))
                        ob = box[0]
                        po = ps_w.tile([P, QW], F32, tag="w", name=f"w{next(_uid)}")
                        oqt, ooff = divmod(st, 4)
                        for pr in range(PAIRS):
                            nc.tensor.matmul(
                                po[:],
                                ctxq[oqt][:, pr, ooff * P : (ooff + 1) * P],
                                wo_sb[:, pr, nt * QW : (nt + 1) * QW],
                                start=(pr == 0),
                                stop=(pr == PAIRS - 1),
                            )
                        osl = slice(nt * QW, (nt + 1) * QW)
                        if tail and (st + nt) % 2 == 1:
                            nc.scalar.copy(ob[:, osl], po[:])
                        else:
                            nc.vector.tensor_copy(ob[:, osl], po[:])
                        if tail:
                            nc.sync.dma_start(
                                out[st * P : (st + 1) * P, osl], ob[:, osl]
                            )
                        elif nt == 1:
                            nc.sync.dma_start(
                                out[st * P : (st + 1) * P, :], ob[:]
                            )

                    return emit

                return [mk(0), mk(1)]

            sched = [(pr, qt, 0, QW)
                     for qt in range(QT_TILES) for pr in (0, 1)]
            sc_stream = [(pr, qt, kc, q0, qw)
                         for (pr, qt, q0, qw) in sched for kc in range(nkc)]
            sc_pos = [0]
            et_map = {}

            def emit_next_sc():
                if sc_pos[0] >= len(sc_stream):
                    return
                pr, qt, kc, q0, qw = sc_stream[sc_pos[0]]
                sc_pos[0] += 1
                qsl = slice(qt * QW + q0, qt * QW + q0 + qw)
                sc = ps_sc.tile([P, 2, QW], F32, tag="sc", name=f"sc{next(_uid)}")
                for hh in range(2):
                    nc.tensor.matmul(
                        sc[:, hh, :qw],
                        KT[hh * DH : (hh + 1) * DH, pr, kc * P : (kc + 1) * P],
                        QT[hh * DH : (hh + 1) * DH, pr, qsl],
                        start=True,
                        stop=True,
                        tile_position=(hh * DH, 0),
                    )
                et = ep.tile([P, 2, QW], BF16, tag="et", name=f"et{next(_uid)}")
                nc.scalar.activation(et[:, :, :qw], sc[:, :, :qw],
                                     AF.Exp, scale=float(SCALE))
                et_map[(pr, qt, kc, q0)] = et

            def attention(pr, qt, fillers, max_pops_per_kc, q0=0, qw=QW,
                          last=False):
                cps = [
                    ps_ctx.tile([DH + 1, QW], F32, tag="ctx", name=f"ctx{hh}")
                    for hh in range(2)
                ]
                budget = min(len(fillers), max_pops_per_kc * nkc)
                if max_pops_per_kc == 1:
                    # keep spare (non-critical) fillers for later attentions,
                    # but always drain through the last correctness-critical
                    # chunk (its write must be emitted before its reader)
                    last_crit = -1
                    for i, (_, crit) in enumerate(fillers):
                        if crit:
                            last_crit = i
                    budget = min(budget, max(6, last_crit + 1))
                popped = 0
                for kc in range(nkc):
                    target = ((kc + 1) * budget + nkc - 1) // nkc
                    while popped < target and fillers:
                        fillers.popleft()[0]()
                        popped += 1
                    emit_next_sc()
                    et = et_map.pop((pr, qt, kc, q0))
                    for hh in range(2):
                        h = 2 * pr + hh
                        nc.tensor.matmul(
                            cps[hh][: DH + 1, :qw],
                            VE[:, kc, h * (DH + 1) : (h + 1) * (DH + 1)],
                            et[:, hh, :qw],
                            start=(kc == 0),
                            stop=(kc == nkc - 1),
                        )
                # fast PSUM evacuation (frees ctx banks), then deferred
                # normalize off the PE critical path. The final attention has
                # no successor waiting on the banks: normalize straight from
                # PSUM and skip the evacuation hop.
                if last:
                    recBs = []
                    for hh in range(2):
                        den = sp.tile([1, QW], F32, tag="den", name=f"den{hh}")
                        nc.vector.tensor_copy(den[:, :qw],
                                              cps[hh][DH : DH + 1, :qw])
                        rec = sp.tile([1, QW], F32, tag="rec", name=f"rec{hh}")
                        nc.vector.reciprocal_approx_fast(rec[:, :qw], den[:, :qw])
                        recB = sp.tile([DH, QW], F32, tag="recB",
                                       name=f"recB{hh}")
                        nc.gpsimd.partition_broadcast(recB[:, :qw], rec[:, :qw])
                        recBs.append(recB)
                    for hh in range(2):
                        nc.vector.tensor_mul(
                            ctxq[qt][hh * DH : (hh + 1) * DH, pr, q0 : q0 + qw],
                            cps[hh][:DH, :qw],
                            recBs[hh][:, :qw],
                        )
                    return
                craws = []
                for hh in range(2):
                    craw = cw.tile([DH + 1, QW], F32, tag="craw",
                                   name=f"craw{hh}")
                    nc.vector.tensor_copy(craw[:, :qw], cps[hh][:, :qw])
                    craws.append(craw)
                for hh in range(2):
                    craw = craws[hh]
                    den = sp.tile([1, QW], F32, tag="den", name=f"den{hh}")
                    nc.vector.tensor_copy(den[:, :qw], craw[DH : DH + 1, :qw])
                    rec = sp.tile([1, QW], F32, tag="rec", name=f"rec{hh}")
                    nc.vector.reciprocal_approx_fast(rec[:, :qw], den[:, :qw])
                    recB = sp.tile([DH, QW], F32, tag="recB", name=f"recB{hh}")
                    nc.gpsimd.partition_broadcast(recB[:, :qw], rec[:, :qw])
                    nc.vector.tensor_mul(
                        ctxq[qt][hh * DH : (hh + 1) * DH, pr, q0 : q0 + qw],
                        craw[:DH, :qw],
                        recB[:, :qw],
                    )

            # ---- emission (scheduling priority) ----
            import os
            _NOFILL = os.environ.get("KMOD_NOFILL") == "1"
            N_PRE_V = nkc if _NOFILL else min(12, nkc)
            for st in range(N_PRE_V):
                for ch in v_chunks(st):
                    ch()
            k_blks = list(range(NKB))
            n_pre_k = NKB if _NOFILL else (2 if KBS == 3 else 4)
            for blk in k_blks[:n_pre_k]:
                for ch in k_chunks(0, blk):
                    ch()
            for ch in q_chunks(0, 0):
                ch()
            for ch in q_chunks(1, 0):
                ch()

            # K(pair0) blocks first (sc(0,0,kc) consumes block kc//KBS with a
            # one-iteration emission lookahead), then trailing V (consumed at
            # ctx(st)), then K(pair1) for the second attention
            def crit(chs):
                return [(f, True) for f in chs]

            def noncrit(chs):
                return [(f, False) for f in chs]

            fillers = deque()
            for blk in k_blks[n_pre_k:]:
                fillers.extend(crit(k_chunks(0, blk)))
            for st in range(N_PRE_V, nkc):
                fillers.extend(crit(v_chunks(st)))
            for blk in k_blks:
                fillers.extend(crit(k_chunks(1, blk)))

            def drain():
                while fillers:
                    fillers.popleft()[0]()

            emit_next_sc()
            for qt in range(QT_TILES):
                if _NOFILL:
                    drain()
                mp0 = -(-len(fillers) // nkc) if qt == 0 else 1
                attention(0, qt, fillers, 0 if _NOFILL else mp0)
                if qt + 1 < QT_TILES:
                    fillers.extend(crit(q_chunks(0, qt + 1)))
                if qt >= 1:
                    for st in ((4 * qt - 2, 4 * qt - 1) if qt < 3 else (10, 11)):
                        fillers.extend(noncrit(out_chunks(st)))
                if _NOFILL:
                    drain()
                attention(1, qt, fillers, 0 if _NOFILL else 1,
                          last=(qt == QT_TILES - 1))
                if qt + 1 < QT_TILES:
                    fillers.extend(crit(q_chunks(1, qt + 1)))
                    for st in (4 * qt, 4 * qt + 1):
                        fillers.extend(noncrit(out_chunks(st)))
            for st in (12, 13, 14, 15):
                fillers.extend(noncrit(out_chunks(st, tail=True)))
            drain()

    nc.finalize()
    return nc


def shard_inputs(x, Wq, bq, Wk, bk, Wv, bv, Wo, bo, mask):
    """Full inputs -> (nkc, list of 8 per-core input maps)."""
    x = np.asarray(x, dtype=np.float32)
    mask = np.asarray(mask)
    kept = [np.flatnonzero(~mask[b]) for b in range(2)]
    nkc = max(1, max((len(k) + P - 1) // P for k in kept))
    NK = nkc * P
    KBS = 3 if nkc % 3 == 0 else 1
    NKB = nkc // KBS
    KW = KBS * P

    def to_T_blocked(a):
        # [rows, cols(=n*128)] fp32 -> [128, n, rows] bf16 with
        # out[p, c, r] = a[r, c*128+p]
        rows, cols = a.shape
        n = cols // P
        return np.ascontiguousarray(
            a.T.astype(NPBF16).reshape(n, P, rows).transpose(1, 0, 2)
        )

    per_batch = {}
    for b in range(2):
        idx = kept[b]
        xk = np.zeros((NK, D), dtype=np.float32)
        xk[: len(idx)] = x[b][idx]
        mc = np.zeros((NK,), dtype=np.float32)
        mc[: len(idx)] = 1.0
        xt_t = to_T_blocked(x[b])        # [P, DCH, S]
        xtk_t = to_T_blocked(xk)         # [P, DCH, NK]
        per_batch[b] = {
            "xt": np.ascontiguousarray(
                xt_t.reshape(P, DCH, QT_TILES, QW).transpose(2, 0, 1, 3)
            ),
            "xtk": np.ascontiguousarray(
                xtk_t.reshape(P, DCH, NKB, KW).transpose(2, 0, 1, 3)
            ),
            "mcol": np.ascontiguousarray(mc.reshape(nkc, P).T),
        }

    ins = []
    for c in range(N_CORES):
        b, g = divmod(c, 4)
        cs = slice(g * 256, (g + 1) * 256)
        wq_h = np.ascontiguousarray(
            Wq[:, cs].astype(NPBF16).reshape(DCH, P, 2 * P).transpose(1, 0, 2)
        )
        wk_h = np.ascontiguousarray(
            Wk[:, cs].astype(NPBF16).reshape(DCH, P, 2 * P).transpose(1, 0, 2)
        )
        wv_h = np.ascontiguousarray(
            Wv[:, cs].astype(NPBF16).reshape(DCH, P, 2 * P).transpose(1, 0, 2)
        )
        wo_h = np.ascontiguousarray(
            Wo[cs, :].astype(NPBF16).reshape(PAIRS, P, D).transpose(1, 0, 2)
        )
        ins.append(
            {
                **per_batch[b],
                "wq": wq_h,
                "wk": wk_h,
                "wv": wv_h,
                "wo": wo_h,
                "bq": np.ascontiguousarray(
                    np.asarray(bq[cs], dtype=np.float32).reshape(PAIRS, P).T
                ),
                "bk": np.ascontiguousarray(
                    np.asarray(bk[cs], dtype=np.float32).reshape(PAIRS, P).T
                ),
                "bvB": np.ascontiguousarray(
                    np.tile(np.asarray(bv[cs], dtype=np.float32)[None, :], (P, 1))
                ),
            }
        )
    return nkc, ins


def gather_outputs(results, bo):
    """8 per-core partial outputs -> full (2, S, D) fp32 output."""
    outs = []
    for b in range(2):
        acc = results[4 * b]["out"].astype(np.float32).copy()
        for g in range(1, 4):
            acc += results[4 * b + g]["out"]
        outs.append(acc + np.asarray(bo, dtype=np.float32))
    return np.stack(outs, axis=0)


_NC_CACHE = {}


def _get_nc(nkc):
    if nkc not in _NC_CACHE:
        _NC_CACHE[nkc] = build(nkc)
    return _NC_CACHE[nkc]


def run_sharded(inputs, trace=False, tmpdir=None):
    """Shard, run on cores 0-7, gather. Returns (output, BassKernelResults)."""
    nkc, ins = shard_inputs(**inputs)
    nc = _get_nc(nkc)
    res = run_bass_kernel_spmd(
        nc, ins, core_ids=list(range(N_CORES)), trace=trace, tmpdir=tmpdir
    )
    full = gather_outputs(res.results, inputs["bo"])
    return full, res


def kernel(**inputs) -> np.ndarray:
    full, _ = run_sharded(inputs, trace=False)
    return full


# revision 34
# speedup vs baseline: 1.0206x; 1.0048x over previous
"""Multi-head attention (B=2, S=2048, D=1024, H=16, Dh=64) on 8 Trainium2
NeuronCores via Bass/Tile.

Sharding: core c = 4*b + g handles batch b and head group g (4 heads =
2 "pairs" of 64-dim heads stacked on the 128-partition dim), with the
matching column/row slices of Wq/Wk/Wv/Wo. Each core returns its partial
output projection; the host sums the 4 partials per batch and adds bo.

Design notes:
  * Host pre-transposes + pre-casts x and the weight slices to bf16 in the
    exact SBUF layouts (no on-device PE transposes, no casting DMAs).
  * The key axis is compacted on host to the unmasked keys (padded to a
    whole number of 128-key chunks): scores/ctx/K-proj/V-proj matmuls and
    the exp() stream all shrink by the masked fraction. Pad keys get V=0
    and a 0 in the denominator column, so no mask arithmetic on device.
  * The attention kc loop is software-pipelined (scores for kc+1 issued
    before ctx for kc) and all remaining projection / output work is
    diced into 2-matmul "filler chunks" popped one (or two) per kc so the
    PE stays busy at the exp-paced pipeline rate without starving the ACT
    engine.
  * Normalization is deferred: ctx PSUM (with the denominator row) is
    evacuated to SBUF with plain copies to free the PSUM bank quickly;
    reciprocal/broadcast/multiply then run off the critical PE path.
  * Weights load on the scalar engine's DMA queue in parallel with x
    tiles on the sync queue; small tensors ride the gpsimd queue.

Per-core math: QT/KT = W^T x^T + b in [dh, s] layout, V_ext =
[(x_kept Wv + bv) * keepmask | keepmask] per head; per q-tile & key chunk:
scT = KT_chunk^T QT_tile (2 heads row-packed in the PE), eT =
exp(SCALE * scT) (one ACT op per pair), ctx_h[65, q] += V_ext^T eT_h
(row 64 = softmax denominator); normalize via reciprocal+broadcast; then
out_partial = ctxT^T Wo_g accumulated over the 2 pairs.
"""

import itertools
import math
from collections import deque

import ml_dtypes
import numpy as np

import concourse.bacc as bacc
import concourse.mybir as mybir
import concourse.tile as tile
from concourse.bass_utils import run_bass_kernel_spmd

F32 = mybir.dt.float32
BF16 = mybir.dt.bfloat16
AF = mybir.ActivationFunctionType
NPBF16 = ml_dtypes.bfloat16

S = 2048
D = 1024
HPC = 4                  # heads per core
DH = 64
PAIRS = 2                # head pairs per core
P = 128
QW = 512                 # q tile width
QT_TILES = S // QW       # 4
DCH = D // P             # 8
SCALE = 1.0 / math.sqrt(DH)

N_CORES = 8


def build(nkc):
    """Build the per-core kernel for `nkc` 128-key chunks of kept keys."""
    _uid = itertools.count()
    NK = nkc * P
    nc = bacc.Bacc(None, target_bir_lowering=False, num_swdge_queues=4)

    xt = nc.dram_tensor("xt", [QT_TILES, P, DCH, QW], BF16, kind="ExternalInput")
    KBS = 3 if nkc % 3 == 0 else 1      # key-block size in 128-key chunks
    NKB = nkc // KBS
    KW = KBS * P
    xtk = nc.dram_tensor("xtk", [NKB, P, DCH, KW], BF16, kind="ExternalInput")
    wq = nc.dram_tensor("wq", [P, DCH, 2 * P], BF16, kind="ExternalInput")
    wk = nc.dram_tensor("wk", [P, DCH, 2 * P], BF16, kind="ExternalInput")
    wv = nc.dram_tensor("wv", [P, DCH, 2 * P], BF16, kind="ExternalInput")
    wo = nc.dram_tensor("wo", [P, PAIRS, D], BF16, kind="ExternalInput")
    bq = nc.dram_tensor("bq", [P, PAIRS], F32, kind="ExternalInput")
    bk = nc.dram_tensor("bk", [P, PAIRS], F32, kind="ExternalInput")
    bvB = nc.dram_tensor("bvB", [P, 2 * P], F32, kind="ExternalInput")
    mcol = nc.dram_tensor("mcol", [P, nkc], F32, kind="ExternalInput")
    mbias = nc.dram_tensor("mbias", [P, nkc], F32, kind="ExternalInput")
    out = nc.dram_tensor("out", [S, D], BF16, kind="ExternalOutput")

    with tile.TileContext(nc) as tc:
        with (
            tc.tile_pool(name="persist", bufs=1) as pp,
            tc.tile_pool(name="vstage", bufs=4) as xs,
            tc.tile_pool(name="expp", bufs=4) as ep,
            tc.tile_pool(name="smalls", bufs=6) as sp,
            tc.tile_pool(name="craws", bufs=2) as cw,
            tc.tile_pool(name="ps_sc", bufs=2, space="PSUM") as ps_sc,
            tc.tile_pool(name="ps_ctx", bufs=2, space="PSUM") as ps_ctx,
            tc.tile_pool(name="ps_w", bufs=2, space="PSUM") as ps_w,
        ):
            # ---- persistent SBUF tensors ----
            wq_sb = pp.tile([P, DCH, 2 * P], BF16)
            wk_sb = pp.tile([P, DCH, 2 * P], BF16)
            wv_sb = pp.tile([P, DCH, 2 * P], BF16)
            wo_sb = pp.tile([P, PAIRS, D], BF16)
            xt_sb = pp.tile([P, QT_TILES, DCH, QW], BF16)
            xtk_sb = pp.tile([P, NKB, DCH, KW], BF16)
            bq_sb = pp.tile([P, PAIRS], F32)
            bk_sb = pp.tile([P, PAIRS], F32)
            bvB_sb = pp.tile([P, 2 * P], F32)
            mcol_sb = pp.tile([P, nkc], F32)
            mbias_sb = pp.tile([P, nkc], F32)

            QT = pp.tile([P, PAIRS, S], BF16)
            KT = pp.tile([P, PAIRS, NK], BF16)
            VE = pp.tile([P, nkc, HPC * (DH + 1)], BF16)
            ctxq = [pp.tile([P, PAIRS, QW], BF16, name=f"ctxq{i}")
                    for i in range(QT_TILES)]

            # ---- input DMAs: smalls on gpsimd, weights on the scalar
            # engine's queue, x tiles on sync (FIFO = priority) ----
            nc.gpsimd.dma_start(mcol_sb[:], mcol[:])
            nc.gpsimd.dma_start(mbias_sb[:], mbias[:])
            nc.gpsimd.dma_start(bvB_sb[:], bvB[:])
            nc.gpsimd.dma_start(bq_sb[:], bq[:])
            nc.gpsimd.dma_start(bk_sb[:], bk[:])
            nc.scalar.dma_start(wv_sb[:], wv[:])
            nc.scalar.dma_start(wk_sb[:], wk[:])
            nc.scalar.dma_start(wq_sb[:], wq[:])
            nc.scalar.dma_start(wo_sb[:], wo[:])
            for wi in range(NKB):
                nc.sync.dma_start(xtk_sb[:, wi, :, :], xtk[wi])
            for qt in range(QT_TILES):
                nc.sync.dma_start(xt_sb[:, qt, :, :], xt[qt])

            # keep-mask (1=kept, 0=pad) into the denominator columns of V_ext
            ve4 = VE[:].rearrange("p k (h c) -> p k h c", h=HPC)
            nc.vector.tensor_copy(
                ve4[:, :, :, DH : DH + 1],
                mcol_sb[:, :, None, None].to_broadcast([P, nkc, HPC, 1]),
            )

            # ---- work units, diced into 2-matmul chunks ----
            def v_chunks(st):
                box = []

                def mk(dcs, final):
                    def emit():
                        if not box:
                            box.append(ps_w.tile([P, QW], F32, tag="w", name=f"w{next(_uid)}"))
                        pv = box[0]
                        blk, off = divmod(st, KBS)
                        for dc in dcs:
                            nc.tensor.matmul(
                                pv[:, : 2 * P],
                                xtk_sb[:, blk, dc, off * P : (off + 1) * P],
                                wv_sb[:, dc, :],
                                start=(dc == 0),
                                stop=(dc == DCH - 1),
                            )
                        if final:
                            nc.vector.tensor_add(
                                ve4[:, st, :, 0:DH],
                                pv[:, : 2 * P].rearrange(
                                    "p (h c) -> p h c", h=HPC
                                ),
                                bvB_sb[:].rearrange("p (h c) -> p h c", h=HPC),
                            )

                    return emit

                return [mk([0, 1], False), mk([2, 3], False),
                        mk([4, 5], False), mk([6, 7], True)]

            def kq_chunks(dst, dsl, src, w_sb, b_sb, pr, width):
                # dst[:, pr, dsl] (width wide) = W_pr^T @ src(dc) + b
                box = []

                def mk(dcs, final):
                    def emit():
                        if not box:
                            box.append(ps_w.tile([P, QW], F32, tag="w", name=f"w{next(_uid)}"))
                        pq = box[0]
                        for dc in dcs:
                            nc.tensor.matmul(
                                pq[:, :width],
                                w_sb[:, dc, pr * P : (pr + 1) * P],
                                src(dc),
                                start=(dc == 0),
                                stop=(dc == DCH - 1),
                            )
                        if final:
                            nc.vector.tensor_scalar_add(
                                dst[:, pr, dsl],
                                pq[:, :width],
                                b_sb[:, pr : pr + 1],
                            )

                    return emit

                return [mk([0, 1], False), mk([2, 3], False),
                        mk([4, 5], False), mk([6, 7], True)]

            def q_chunks(pr, qt):
                return kq_chunks(
                    QT, slice(qt * QW, (qt + 1) * QW),
                    lambda dc: xt_sb[:, qt, dc, :],
                    wq_sb, bq_sb, pr, QW,
                )

            def k_chunks(pr, blk):
                return kq_chunks(
                    KT, slice(blk * KW, (blk + 1) * KW),
                    lambda dc: xtk_sb[:, blk, dc, :],
                    wk_sb, bk_sb, pr, KW,
                )

            def out_chunks(st, tail=False):
                box = []

                def mk(nt):
                    def emit():
                        if not box:
                            box.append(xs.tile([P, D], BF16, tag="ob", name=f"ob{next(_uid)}"))
                        ob = box[0]
                        po = ps_w.tile([P, QW], F32, tag="w", name=f"w{next(_uid)}")
                        oqt, ooff = divmod(st, 4)
                        for pr in range(PAIRS):
                            nc.tensor.matmul(
                                po[:],
                                ctxq[oqt][:, pr, ooff * P : (ooff + 1) * P],
                                wo_sb[:, pr, nt * QW : (nt + 1) * QW],
                                start=(pr == 0),
                                stop=(pr == PAIRS - 1),
                            )
                        osl = slice(nt * QW, (nt + 1) * QW)
                        if tail and (st + nt) % 2 == 1:
                            nc.scalar.copy(ob[:, osl], po[:])
                        else:
                            nc.vector.tensor_copy(ob[:, osl], po[:])
                        if tail:
                            nc.sync.dma_start(
                                out[st * P : (st + 1) * P, osl], ob[:, osl]
                            )
                        elif nt == 1:
                            nc.sync.dma_start(
                                out[st * P : (st + 1) * P, :], ob[:]
                            )

                    return emit

                return [mk(0), mk(1)]

            sched = [(pr, qt, 0, QW)
                     for qt in range(QT_TILES) for pr in (0, 1)]
            sc_stream = [(pr, qt, kc, q0, qw)
                         for (pr, qt, q0, qw) in sched for kc in range(nkc)]
            sc_pos = [0]
            et_map = {}

            def emit_next_sc():
                if sc_pos[0] >= len(sc_stream):
                    return
                pr, qt, kc, q0, qw = sc_stream[sc_pos[0]]
                sc_pos[0] += 1
                qsl = slice(qt * QW + q0, qt * QW + q0 + qw)
                sc = ps_sc.tile([P, 2, QW], F32, tag="sc", name=f"sc{next(_uid)}")
                for hh in range(2):
                    nc.tensor.matmul(
                        sc[:, hh, :qw],
                        KT[hh * DH : (hh + 1) * DH, pr, kc * P : (kc + 1) * P],
                        QT[hh * DH : (hh + 1) * DH, pr, qsl],
                        start=True,
                        stop=True,
                        tile_position=(hh * DH, 0),
                    )
                et = ep.tile([P, 2, QW], BF16, tag="et", name=f"et{next(_uid)}")
                nc.scalar.activation(et[:, :, :qw], sc[:, :, :qw],
                                     AF.Exp, scale=float(SCALE),
                                     bias=mbias_sb[:, kc : kc + 1])
                et_map[(pr, qt, kc, q0)] = et

            def attention(pr, qt, fillers, max_pops_per_kc, q0=0, qw=QW,
                          last=False):
                cps = [
                    ps_ctx.tile([DH + 1, QW], F32, tag="ctx", name=f"ctx{hh}")
                    for hh in range(2)
                ]
                budget = min(len(fillers), max_pops_per_kc * nkc)
                if max_pops_per_kc == 1:
                    # keep spare (non-critical) fillers for later attentions,
                    # but always drain through the last correctness-critical
                    # chunk (its write must be emitted before its reader)
                    last_crit = -1
                    for i, (_, crit) in enumerate(fillers):
                        if crit:
                            last_crit = i
                    budget = min(budget, max(6, last_crit + 1))
                popped = 0
                for kc in range(nkc):
                    target = ((kc + 1) * budget + nkc - 1) // nkc
                    while popped < target and fillers:
                        fillers.popleft()[0]()
                        popped += 1
                    emit_next_sc()
                    et = et_map.pop((pr, qt, kc, q0))
                    for hh in range(2):
                        h = 2 * pr + hh
                        nc.tensor.matmul(
                            cps[hh][: DH + 1, :qw],
                            VE[:, kc, h * (DH + 1) : (h + 1) * (DH + 1)],
                            et[:, hh, :qw],
                            start=(kc == 0),
                            stop=(kc == nkc - 1),
                        )
                # fast PSUM evacuation (frees ctx banks), then deferred
                # normalize off the PE critical path. The final attention has
                # no successor waiting on the banks: normalize straight from
                # PSUM and skip the evacuation hop.
                if last:
                    recBs = []
                    for hh in range(2):
                        den = sp.tile([1, QW], F32, tag="den", name=f"den{hh}")
                        nc.vector.tensor_copy(den[:, :qw],
                                              cps[hh][DH : DH + 1, :qw])
                        rec = sp.tile([1, QW], F32, tag="rec", name=f"rec{hh}")
                        nc.vector.reciprocal_approx_fast(rec[:, :qw], den[:, :qw])
                        recB = sp.tile([DH, QW], F32, tag="recB",
                                       name=f"recB{hh}")
                        nc.gpsimd.partition_broadcast(recB[:, :qw], rec[:, :qw])
                        recBs.append(recB)
                    for hh in range(2):
                        nc.vector.tensor_mul(
                            ctxq[qt][hh * DH : (hh + 1) * DH, pr, q0 : q0 + qw],
                            cps[hh][:DH, :qw],
                            recBs[hh][:, :qw],
                        )
                    return
                craws = []
                for hh in range(2):
                    craw = cw.tile([DH + 1, QW], F32, tag="craw",
                                   name=f"craw{hh}")
                    nc.vector.tensor_copy(craw[:, :qw], cps[hh][:, :qw])
                    craws.append(craw)
                for hh in range(2):
                    craw = craws[hh]
                    den = sp.tile([1, QW], F32, tag="den", name=f"den{hh}")
                    nc.vector.tensor_copy(den[:, :qw], craw[DH : DH + 1, :qw])
                    rec = sp.tile([1, QW], F32, tag="rec", name=f"rec{hh}")
                    nc.vector.reciprocal_approx_fast(rec[:, :qw], den[:, :qw])
                    recB = sp.tile([DH, QW], F32, tag="recB", name=f"recB{hh}")
                    nc.gpsimd.partition_broadcast(recB[:, :qw], rec[:, :qw])
                    nc.vector.tensor_mul(
                        ctxq[qt][hh * DH : (hh + 1) * DH, pr, q0 : q0 + qw],
                        craw[:DH, :qw],
                        recB[:, :qw],
                    )

            # ---- emission (scheduling priority) ----
            import os
            _NOFILL = os.environ.get("KMOD_NOFILL") == "1"
            N_PRE_V = nkc if _NOFILL else min(12, nkc)
            for st in range(N_PRE_V):
                for ch in v_chunks(st):
                    ch()
            k_blks = list(range(NKB))
            n_pre_k = NKB if _NOFILL else (2 if KBS == 3 else 4)
            for blk in k_blks[:n_pre_k]:
                for ch in k_chunks(0, blk):
                    ch()
            for ch in q_chunks(0, 0):
                ch()
            for ch in q_chunks(1, 0):
                ch()

            # K(pair0) blocks first (sc(0,0,kc) consumes block kc//KBS with a
            # one-iteration emission lookahead), then trailing V (consumed at
            # ctx(st)), then K(pair1) for the second attention
            def crit(chs):
                return [(f, True) for f in chs]

            def noncrit(chs):
                return [(f, False) for f in chs]

            fillers = deque()
            for blk in k_blks[n_pre_k:]:
                fillers.extend(crit(k_chunks(0, blk)))
            for st in range(N_PRE_V, nkc):
                fillers.extend(crit(v_chunks(st)))
            for blk in k_blks:
                fillers.extend(crit(k_chunks(1, blk)))

            def drain():
                while fillers:
                    fillers.popleft()[0]()

            emit_next_sc()
            for qt in range(QT_TILES):
                if _NOFILL:
                    drain()
                mp0 = -(-len(fillers) // nkc) if qt == 0 else 1
                attention(0, qt, fillers, 0 if _NOFILL else mp0)
                if qt + 1 < QT_TILES:
                    fillers.extend(crit(q_chunks(0, qt + 1)))
                if qt >= 1:
                    for st in ((4 * qt - 2, 4 * qt - 1) if qt < 3 else (10, 11)):
                        fillers.extend(noncrit(out_chunks(st)))
                if _NOFILL:
                    drain()
                attention(1, qt, fillers, 0 if _NOFILL else 1,
                          last=(qt == QT_TILES - 1))
                if qt + 1 < QT_TILES:
                    fillers.extend(crit(q_chunks(1, qt + 1)))
                    for st in (4 * qt, 4 * qt + 1):
                        fillers.extend(noncrit(out_chunks(st)))
            for st in (12, 13, 14, 15):
                fillers.extend(noncrit(out_chunks(st, tail=True)))
            drain()

    nc.finalize()
    return nc


def shard_inputs(x, Wq, bq, Wk, bk, Wv, bv, Wo, bo, mask):
    """Full inputs -> (nkc, list of 8 per-core input maps)."""
    x = np.asarray(x, dtype=np.float32)
    mask = np.asarray(mask)
    kept = [np.flatnonzero(~mask[b]) for b in range(2)]
    nkc = max(1, max((len(k) + P - 1) // P for k in kept))
    NK = nkc * P
    KBS = 3 if nkc % 3 == 0 else 1
    NKB = nkc // KBS
    KW = KBS * P

    def to_T_blocked(a):
        # [rows, cols(=n*128)] fp32 -> [128, n, rows] bf16 with
        # out[p, c, r] = a[r, c*128+p]
        rows, cols = a.shape
        n = cols // P
        return np.ascontiguousarray(
            a.T.astype(NPBF16).reshape(n, P, rows).transpose(1, 0, 2)
        )

    per_batch = {}
    for b in range(2):
        idx = kept[b]
        xk = np.zeros((NK, D), dtype=np.float32)
        xk[: len(idx)] = x[b][idx]
        mc = np.zeros((NK,), dtype=np.float32)
        mc[: len(idx)] = 1.0
        xt_t = to_T_blocked(x[b])        # [P, DCH, S]
        xtk_t = to_T_blocked(xk)         # [P, DCH, NK]
        per_batch[b] = {
            "xt": np.ascontiguousarray(
                xt_t.reshape(P, DCH, QT_TILES, QW).transpose(2, 0, 1, 3)
            ),
            "xtk": np.ascontiguousarray(
                xtk_t.reshape(P, DCH, NKB, KW).transpose(2, 0, 1, 3)
            ),
            "mcol": np.ascontiguousarray(mc.reshape(nkc, P).T),
            "mbias": np.ascontiguousarray(
                ((mc - 1.0) * 30.0).reshape(nkc, P).T
            ),
        }

    ins = []
    for c in range(N_CORES):
        b, g = divmod(c, 4)
        cs = slice(g * 256, (g + 1) * 256)
        wq_h = np.ascontiguousarray(
            Wq[:, cs].astype(NPBF16).reshape(DCH, P, 2 * P).transpose(1, 0, 2)
        )
        wk_h = np.ascontiguousarray(
            Wk[:, cs].astype(NPBF16).reshape(DCH, P, 2 * P).transpose(1, 0, 2)
        )
        wv_h = np.ascontiguousarray(
            Wv[:, cs].astype(NPBF16).reshape(DCH, P, 2 * P).transpose(1, 0, 2)
        )
        wo_h = np.ascontiguousarray(
            Wo[cs, :].astype(NPBF16).reshape(PAIRS, P, D).transpose(1, 0, 2)
        )
        ins.append(
            {
                **per_batch[b],
                "wq": wq_h,
                "wk": wk_h,
                "wv": wv_h,
                "wo": wo_h,
                "bq": np.ascontiguousarray(
                    np.asarray(bq[cs], dtype=np.float32).reshape(PAIRS, P).T
                ),
                "bk": np.ascontiguousarray(
                    np.asarray(bk[cs], dtype=np.float32).reshape(PAIRS, P).T
                ),
                "bvB": np.ascontiguousarray(
                    np.tile(np.asarray(bv[cs], dtype=np.float32)[None, :], (P, 1))
                ),
            }
        )
    return nkc, ins


def gather_outputs(results, bo):
    """8 per-core partial outputs -> full (2, S, D) fp32 output."""
    outs = []
    for b in range(2):
        acc = results[4 * b]["out"].astype(np.float32).copy()
        for g in range(1, 4):
            acc += results[4 * b + g]["out"]
        outs.append(acc + np.asarray(bo, dtype=np.float32))
    return np.stack(outs, axis=0)


_NC_CACHE = {}


def _get_nc(nkc):
    if nkc not in _NC_CACHE:
        _NC_CACHE[nkc] = build(nkc)
    return _NC_CACHE[nkc]


def run_sharded(inputs, trace=False, tmpdir=None):
    """Shard, run on cores 0-7, gather. Returns (output, BassKernelResults)."""
    nkc, ins = shard_inputs(**inputs)
    nc = _get_nc(nkc)
    res = run_bass_kernel_spmd(
        nc, ins, core_ids=list(range(N_CORES)), trace=trace, tmpdir=tmpdir
    )
    full = gather_outputs(res.results, inputs["bo"])
    return full, res


def kernel(**inputs) -> np.ndarray:
    full, _ = run_sharded(inputs, trace=False)
    return full
